# revision 1
# baseline (speedup 1.0000x reference)
"""MoE balancing-loss kernel for Trainium2 (8 NeuronCores, data-parallel over tokens).

Problem: router_logits [32, 16384, 64] f32 ->
    loss = 0.01 * sum_l (E/(T*K)) * sum_e counts[l,e] * mean_t(softmax(logits)[l,t,e])
where counts[l,e] = #tokens whose top-8 (by softmax == by logits) includes expert e.

Sharding: tokens (dim 1) split across 8 cores, 2048 tokens/core. Each core
computes partial counts[l,e] and partial sum_t softmax[l,t,e]; host reduces the
tiny per-layer partials and forms the loss (the global-average all-reduce).

Per-core layout (per layer): one SBUF tile [128 partitions x 1024] f32 where
partition p holds 16 consecutive tokens (slots j=0..15) of 64 logits each.
  ACT : e = exp(x) -> bf16 (no max-subtract needed: |x| <~ 6 for randn inputs)
  DVE : 16x max8 (threshold theta = 8th largest per token), one segmented
        reduce_sum for softmax denominators s[p,j], reciprocal -> bf16 r=1/s,
        one broadcast tensor_tensor is_ge -> bf16 mask
  PE  : rwsum-junk = R^T @ e_half (R [128,16] = r; out [16,512] per half; the
        64-col block at row j is slot j's rwsum partial, rest is junk filtered
        on host); counts = ones^T @ mask_half, both halves PSUM-accumulated
        into [1,512] (slot-blocks folded pairwise on device).
        Two layers stack into each PSUM tile at partition offsets 0/64 (matmul
        output base partition must be one of {0,32,64}).
  out : per layer pair one merged [128, 1536] bf16 staging copy (ACT) and two
        [16, 1536] DMAs (gpsimd queue); host extracts diagonal blocks, sums
        the tiny [32,64] partials over slots and cores, and forms the loss.
"""

import numpy as np

L, T, E = 32, 16384, 64
K = 8
NCORES = 8
TC = T // NCORES          # 2048 tokens per core
P = 128                   # partitions
J = TC // P               # 16 token slots per partition
HF = J * E // 2           # 512, half the free width (PSUM bank limit)
LOSS_WEIGHT = 0.01

_cached = {}


def _build():
    import concourse.bacc as bacc
    import concourse.mybir as mybir
    from concourse.tile import TileContext

    f32 = mybir.dt.float32
    bf16 = mybir.dt.bfloat16
    Alu = mybir.AluOpType

    NPAIR = L // 2    # 2 layers stacked per psum tile at partition 0 / 64

    nc = bacc.Bacc(trn_type="TRN2")
    x = nc.dram_tensor("x", [L, P, J * E], f32, kind="ExternalInput")
    # merged junk output per (pair, layer-in-pair): 16 slot rows x
    # [rw h=0 (512) | rw h=1 (512) | counts (512, row 0 only)] bf16
    out_o = nc.dram_tensor(
        "out_o", [NPAIR, 2, J, 3 * HF], bf16, kind="ExternalOutput"
    )

    with TileContext(nc) as tc:
        with (
            tc.tile_pool(name="const", bufs=1) as cpool,
            tc.tile_pool(name="work", bufs=4) as pool,
            tc.tile_pool(name="psg", bufs=2, space="PSUM") as pgpool,
            tc.tile_pool(name="psc", bufs=1, space="PSUM") as pcpool,
            tc.tile_pool(name="outs", bufs=2) as opool,
        ):
            ones_bf = cpool.tile([P, 1], bf16)
            nc.vector.memset(ones_bf[:], 1.0)

            rw_ps = None
            cnt_ps = None
            for l in range(L):
                pg, li = divmod(l, 2)
                if li == 0:
                    rw_ps = [
                        pgpool.tile([P, HF], f32, tag=f"rw{h}", name=f"rw{h}")
                        for h in range(2)
                    ]
                    cnt_ps = pcpool.tile([P, HF], f32, tag="cnt", name="cnt")
                x_t = pool.tile([P, J * E], f32, tag="x")
                nc.sync.dma_start(x_t[:], x[l])
                x3d = x_t[:].rearrange("p (j e) -> p j e", e=E)

                # exp -> bf16 (ACT, runs in parallel with the max8 chain)
                e_t = pool.tile([P, J * E], bf16, tag="e")
                nc.scalar.activation(
                    e_t[:], x_t[:], mybir.ActivationFunctionType.Exp
                )

                # DVE op order: layer 0 runs max8 first (needs only x_t, so
                # DVE doesn't stall on ACT at pipeline start); later layers
                # run reduce/recip first so the rwsum matmuls + staging copies
                # of the final pair overlap the last max8/TT burst (shorter
                # kernel tail).
                th_t = pool.tile([P, J * 8], f32, tag="th")
                mask_t = pool.tile([P, J * E], bf16, tag="mask")
                s_t = pool.tile([P, J], f32, tag="s")
                r_bf = pool.tile([P, J], bf16, tag="rbf")

                def do_max8_mask():
                    for j in range(J):
                        nc.vector.max(
                            out=th_t[:, j * 8 : (j + 1) * 8],
                            in_=x_t[:, j * E : (j + 1) * E],
                        )
                    th_b = (
                        th_t[:]
                        .rearrange("p (j e) -> p j e", e=8)[:, :, 7:8]
                        .to_broadcast([P, J, E])
                    )
                    nc.vector.tensor_tensor(
                        mask_t[:].rearrange("p (j e) -> p j e", e=E),
                        x3d,
                        th_b,
                        Alu.is_ge,
                    )

                def do_denom():
                    nc.vector.reduce_sum(
                        s_t[:],
                        e_t[:].rearrange("p (j e) -> p j e", e=E),
                        axis=mybir.AxisListType.X,
                    )
                    with nc.allow_low_precision(reason="r is bf16 anyway"):
                        nc.vector.reciprocal(r_bf[:], s_t[:])

                if l == 0:
                    do_max8_mask()
                    do_denom()
                else:
                    do_denom()
                    do_max8_mask()

                # PE: rwsum junk [16, 512] per half at partition 64*li;
                # counts: both halves PSUM-accumulated into [1, 512] at
                # partition 64*li (folds slot-blocks pairwise on device).
                po = 64 * li
                for h in range(2):
                    nc.tensor.matmul(
                        rw_ps[h][po : po + J, :],
                        r_bf[:, :],
                        e_t[:, h * HF : (h + 1) * HF],
                        start=True,
                        stop=True,
                    )
                    nc.tensor.matmul(
                        cnt_ps[po : po + 1, :],
                        ones_bf[:, 0:1],
                        mask_t[:, h * HF : (h + 1) * HF],
                        start=(h == 0),
                        stop=(h == 1),
                    )

                if li == 1:
                    # flush pair: PSUM -> one merged SBUF staging tile (ACT
                    # full-width copies), then one DMA per layer-in-pair
                    ot = opool.tile([P, 3 * HF], bf16, tag="ostg", name="ostg")
                    for h in range(2):
                        nc.scalar.copy(ot[:, h * HF : (h + 1) * HF], rw_ps[h][:, :])
                    nc.scalar.copy(ot[:, 2 * HF : 3 * HF], cnt_ps[:, :])
                    # last pair: use the (idle-by-then) sync queue so the
                    # final transfers don't queue behind earlier gpsimd DMAs
                    q = nc.sync if pg == L // 2 - 1 else nc.gpsimd
                    q.dma_start(out_o[pg, 0], ot[0:J, :])
                    q.dma_start(out_o[pg, 1], ot[64 : 64 + J, :])

    nc.finalize()
    return nc


def _get_nc():
    if "nc" not in _cached:
        _cached["nc"] = _build()
    return _cached["nc"]


def kernel(router_logits, n_routed_experts=E, num_experts_per_tok=K):
    from concourse.bass_utils import run_bass_kernel_spmd

    xl = np.asarray(router_logits, dtype=np.float32)
    assert xl.shape == (L, T, E), xl.shape
    assert int(n_routed_experts) == E and int(num_experts_per_tok) == K

    nc = _get_nc()
    in_maps = []
    for c in range(NCORES):
        sl = np.ascontiguousarray(xl[:, c * TC : (c + 1) * TC, :])
        in_maps.append({"x": sl.reshape(L, P, J * E)})

    try:
        res = run_bass_kernel_spmd(nc, in_maps, core_ids=list(range(NCORES)))
    except Exception:
        # the axon/NRT path occasionally reports the device unrecoverable on
        # the first touch after an earlier crashed process; one retry clears it
        res = run_bass_kernel_spmd(nc, in_maps, core_ids=list(range(NCORES)))

    NPAIR = L // 2
    rwsum = np.zeros((L, E), np.float64)
    counts = np.zeros((L, E), np.float64)
    for c in range(NCORES):
        o = np.asarray(res.results[c]["out_o"]).astype(np.float64)
        # o: [pair, li, slot j (16), 3*512]; cols [512h, 512h+512) hold the
        # rw junk for half h: slot j's rwsum at 512*(j//8) + 64*(j%8) + e.
        # cols [1024, 1536) row 0 hold counts (slot-blocks folded pairwise).
        rw = o[:, :, :, : 2 * HF].reshape(NPAIR, 2, J, 2, 8, E)
        for j in range(J):
            h, jb = divmod(j, 8)
            rwsum += rw[:, :, j, h, jb, :].reshape(L, E)
        counts += (
            o[:, :, 0, 2 * HF :].reshape(NPAIR, 2, 8, E).sum(axis=2).reshape(L, E)
        )

    scale = E / (T * K)
    rw_mean = rwsum / T
    loss = (scale * (counts * rw_mean).sum(-1)).sum() * LOSS_WEIGHT
    return np.float32(loss)

